# revision 9
# baseline (speedup 1.0000x reference)
"""Trainium2 Bass kernel for nn_GunnarODE: neural CDE with hermite spline control.

Contract: kernel(**inputs) takes FULL unsharded inputs (ts, us, ys, W1, b1,
W2, b2, batch_size) and returns the FULL (B, L, Y) output. Internally shards
the batch across 8 NeuronCores (pure data parallel), runs a Bass/Tile kernel
per core, and reassembles.

Algorithm notes (derived from the reference):
  - x = concat([t, us]) with unit-spaced knots (ts is arange) => dt == 1.
  - Hermite backward-difference spline derivative at substep s_i = i/4 of
    interval k reduces to dXdt_i = alpha_i * slope_{k-1} + beta_i * slope_k;
    the time channel has dXdt == 1. The 128-row broadcast of dXdt (row
    r=(c-1)*16+y -> channel c) is precomputed on the host and DMA'd, which
    removes one matmul + serial weight load per substep from the PE.
  - Per Euler substep: h = tanh(z@W1.T+b1); vf = tanh(h@W2.T+b2) viewed as
    (Y=16, C=9); z += 0.25 * einsum(vf, dXdt).
  - Feature-on-partition layout. The 144 vf rows split into 128 "ctrl" rows
    (r=(c-1)*16+y for channels c=1..8) and 16 "time" rows (y*9).
  - hpre = W1 @ z is THE state, held in a persistent fp32 PSUM accumulator;
    per substep it is incremented by two accumulating matmuls, and z is
    reconstructed per interval via R = pinv(W1) for output only.

Precision schedule (validated by a CPU error-amplification study + HW
microbenches):
  - The ODE is chaotic: a perturbation at interval k is amplified by G(k)
    (G(0)~2e4, G(128)~48, G(320)~4). Three bands: fp32 matmuls for k < 128,
    fp16 for 128 <= k < 320, bf16 for k >= 320 (fp32 PSUM accumulate
    everywhere). Simulated end-to-end rel-L2 vs the fp32 reference ~9.4e-3
    (budget 2e-2; the measured 2-band fp16 variant ran at 7.6e-3).
  - PE timing (HW-measured): fp32/fp16 matmuls are ~2 cyc/col; bf16 is
    ~1 cyc/col. Full-width (N=512) bodies beat half-split ones because the
    per-matmul serial LDWEIGHTS and fixed overheads amortize over 2x the
    columns.
"""
import sys
if '/opt/trn_rl_repo' not in sys.path:
    sys.path.insert(0, '/opt/trn_rl_repo')

import numpy as np

N_CORES = 8
L = 512
B_TOT = 4096
U = 8
Y = 16
H = 128
C = U + 1
NI = L - 1            # intervals
HSTEP = 0.25          # dt / SUBSTEPS with dt == 1
B_LOC = B_TOT // N_CORES  # 512
K0 = 128              # first fp16 interval
K1 = 320              # first bf16 interval

ALPHA = [1.0, 0.1875, -0.25, -0.3125]
BETA = [0.0, 0.8125, 1.25, 1.3125]

_BUILD_CACHE = {}


def _host_constants(W1, b1, W2, b2):
    """Precompute transposed/permuted constant matrices (host-side, free)."""
    rowmap = np.array([(r % 16) * 9 + (r // 16 + 1) for r in range(128)])
    cst = {}
    cst["W1T"] = np.ascontiguousarray(W1.T)                        # (16,128)
    cst["W2aT"] = np.ascontiguousarray(W2[rowmap, :].T)            # (128,128)
    cst["W2bT"] = np.ascontiguousarray(W2[np.arange(16) * 9, :].T)  # (128,16)
    cst["b1c"] = np.ascontiguousarray(b1[:, None])                 # (128,1)
    cst["b2c"] = np.ascontiguousarray(b2[rowmap][:, None])         # (128,1)
    cst["b2t"] = np.ascontiguousarray(b2[np.arange(16) * 9][:, None])  # (16,1)
    # replicated time-channel head: vfb[r] = vft[r % 16]
    rep = np.arange(128) % 16
    cst["W2bRT"] = np.ascontiguousarray(cst["W2bT"][:, rep])           # (128,128)
    cst["b2tb"] = np.ascontiguousarray(b2[np.arange(16) * 9][rep][:, None])
    # hpre-state update matrices: hpre += (h*W1*Sel^T) @ tmp + (h*W1) @ vft
    w1selt = np.zeros((128, 128), dtype=np.float32)  # [r, j] = h*W1[j, r%16]
    for r in range(128):
        w1selt[r, :] = HSTEP * W1[:, r % 16]
    cst["W1SelT"] = w1selt / 8.0  # dX is host-prescaled by 8; vfb unscaled
    cst["W1hT"] = (HSTEP * W1.T)                                   # (16,128)
    # output reconstruction: z = pinv(W1) @ hpre  (W1 is 128x16, cond ~2)
    R = np.linalg.pinv(W1.astype(np.float64)).astype(np.float32)   # (16,128)
    cst["RT"] = np.ascontiguousarray(R.T)                          # (128,16)
    return {k: v.astype(np.float32) for k, v in cst.items()}


def _band_sizes(n_intervals, k0=K0, k1=K1):
    n32 = min(k0, n_intervals)
    n16 = min(k1, n_intervals) - n32
    nb8 = n_intervals - n32 - n16
    return n32, n16, nb8


def _build(n_intervals=NI):
    """Build + compile the Bass module (cached per interval count)."""
    key = n_intervals
    if key in _BUILD_CACHE:
        return _BUILD_CACHE[key]

    import concourse.bass as bass
    import concourse.bacc as bacc
    import concourse.tile as tile
    from concourse import mybir

    F32 = mybir.dt.float32
    F32R = mybir.dt.float32r
    F16 = mybir.dt.float16
    BF16 = mybir.dt.bfloat16
    TANH = mybir.ActivationFunctionType.Tanh
    MULT = mybir.AluOpType.mult
    ADD = mybir.AluOpType.add

    n32, n16, nb8 = _band_sizes(n_intervals)

    nc = bacc.Bacc("TRN2", target_bir_lowering=False, debug=False,
                   num_devices=N_CORES)

    # host-precomputed 128-row dXdt broadcasts, one tensor per band
    if n32:
        d_dXa = nc.dram_tensor("dXa", (n32 * 4, 128, B_LOC), F32,
                               kind="ExternalInput")
    if n16:
        d_dXh = nc.dram_tensor("dXh", (n16 * 4, 128, B_LOC), F16,
                               kind="ExternalInput")
    if nb8:
        d_dXw = nc.dram_tensor("dXw", (nb8 * 4, 128, B_LOC), BF16,
                               kind="ExternalInput")
    d_ys0 = nc.dram_tensor("ys0T", (16, B_LOC), F32, kind="ExternalInput")
    d_W1T = nc.dram_tensor("W1T", (16, 128), F32, kind="ExternalInput")
    d_W2aT = nc.dram_tensor("W2aT", (128, 128), F32, kind="ExternalInput")
    d_W2bT = nc.dram_tensor("W2bT", (128, 128), F32, kind="ExternalInput")
    d_b1 = nc.dram_tensor("b1c", (128, 1), F32, kind="ExternalInput")
    d_b2c = nc.dram_tensor("b2c", (128, 1), F32, kind="ExternalInput")
    d_b2t = nc.dram_tensor("b2t", (128, 1), F32, kind="ExternalInput")
    d_W1SelT = nc.dram_tensor("W1SelT", (128, 128), F32, kind="ExternalInput")
    d_W1hT = nc.dram_tensor("W1hT", (16, 128), F32, kind="ExternalInput")
    d_RT = nc.dram_tensor("RT", (128, 16), F32, kind="ExternalInput")
    d_W2aTh = nc.dram_tensor("W2aTh", (128, 128), F16, kind="ExternalInput")
    d_W2bTh = nc.dram_tensor("W2bTh", (128, 128), F16, kind="ExternalInput")
    d_W1SelTh = nc.dram_tensor("W1SelTh", (128, 128), F16, kind="ExternalInput")
    d_W1hTh = nc.dram_tensor("W1hTh", (16, 128), F16, kind="ExternalInput")
    d_W2aTw = nc.dram_tensor("W2aTw", (128, 128), BF16, kind="ExternalInput")
    d_W2bTw = nc.dram_tensor("W2bTw", (128, 128), BF16, kind="ExternalInput")
    d_W1SelTw = nc.dram_tensor("W1SelTw", (128, 128), BF16, kind="ExternalInput")
    d_W1hTw = nc.dram_tensor("W1hTw", (16, 128), BF16, kind="ExternalInput")
    d_out = nc.dram_tensor("out", (n_intervals, 16, B_LOC), F32, kind="ExternalOutput")

    with tile.TileContext(nc) as tc:
        with (
            tc.tile_pool(name="consts", bufs=1) as consts,
            tc.tile_pool(name="zpool", bufs=3) as zpool,
            tc.tile_pool(name="work", bufs=2) as work,
            tc.tile_pool(name="dxp", bufs=3) as dxp,
            tc.tile_pool(name="ps1", bufs=1, space="PSUM") as ps1,
            tc.tile_pool(name="ps2", bufs=2, space="PSUM") as ps2,
        ):
            W1T = consts.tile([16, 128], F32)
            W2aT = consts.tile([128, 128], F32)
            W2bT = consts.tile([128, 128], F32)
            b1c = consts.tile([128, 1], F32)
            b2c = consts.tile([128, 1], F32)
            b2t = consts.tile([128, 1], F32)
            W1SelT = consts.tile([128, 128], F32)
            W1hT = consts.tile([16, 128], F32)
            RTr = consts.tile([128, 16], F32R)
            W2aTh = consts.tile([128, 128], F16)
            W2bTh = consts.tile([128, 128], F16)
            W1SelTh = consts.tile([128, 128], F16)
            W1hTh = consts.tile([16, 128], F16)
            W2aTw = consts.tile([128, 128], BF16)
            W2bTw = consts.tile([128, 128], BF16)
            W1SelTw = consts.tile([128, 128], BF16)
            W1hTw = consts.tile([16, 128], BF16)
            nc.sync.dma_start(W1T[:], d_W1T.ap())
            nc.sync.dma_start(W2aT[:], d_W2aT.ap())
            nc.sync.dma_start(W2bT[:], d_W2bT.ap())
            nc.sync.dma_start(b1c[:], d_b1.ap())
            nc.sync.dma_start(b2c[:], d_b2c.ap())
            nc.sync.dma_start(b2t[:], d_b2t.ap())
            nc.sync.dma_start(W1SelT[:], d_W1SelT.ap())
            nc.sync.dma_start(W1hT[:], d_W1hT.ap())
            nc.sync.dma_start(RTr[:], d_RT.ap().bitcast(F32R))
            nc.sync.dma_start(W2aTh[:], d_W2aTh.ap())
            nc.sync.dma_start(W2bTh[:], d_W2bTh.ap())
            nc.sync.dma_start(W1SelTh[:], d_W1SelTh.ap())
            nc.sync.dma_start(W1hTh[:], d_W1hTh.ap())
            nc.sync.dma_start(W2aTw[:], d_W2aTw.ap())
            nc.sync.dma_start(W2bTw[:], d_W2bTw.ap())
            nc.sync.dma_start(W1SelTw[:], d_W1SelTw.ap())
            nc.sync.dma_start(W1hTw[:], d_W1hTw.ap())

            z0 = zpool.tile([16, B_LOC], F32, tag="z")
            nc.sync.dma_start(z0[:], d_ys0.ap())

            # hpre is THE state: a persistent PSUM accumulator holding W1 @ z.
            hpre = ps1.tile([128, B_LOC], F32, tag="hpre")
            nc.tensor.matmul(hpre[:], W1T[:], z0[:], start=True, stop=False,
                             skip_group_check=True)

            dxs = {}

            def load_dX(k):
                """Prefetch the 4 substep dXdt tiles of interval k."""
                if k >= n_intervals:
                    return
                for i in range(4):
                    if k < n32:
                        t = dxp.tile([128, B_LOC], F32, tag=f"dxa{i}",
                                     name=f"dX_{k}_{i}")
                        nc.sync.dma_start(t[:], d_dXa.ap()[k * 4 + i])
                    elif k < n32 + n16:
                        t = dxp.tile([128, B_LOC], F16, tag=f"dxh{i}",
                                     name=f"dX_{k}_{i}")
                        nc.sync.dma_start(t[:], d_dXh.ap()[(k - n32) * 4 + i])
                    else:
                        t = dxp.tile([128, B_LOC], BF16, tag=f"dxw{i}",
                                     name=f"dX_{k}_{i}")
                        nc.sync.dma_start(t[:], d_dXw.ap()[(k - n32 - n16) * 4 + i])
                    dxs[(k, i)] = t

            def out_interval(k):
                # per-interval output: z_{k+1} = pinv(W1) @ hpre (fp32r is
                # fine: output tap only, no feedback into the state)
                hps = work.tile([128, B_LOC], F32R, tag="hps")
                nc.vector.tensor_copy(hps[:], hpre[:])
                zt_ps = ps2.tile([16, B_LOC], F32, tag="ztp")
                nc.tensor.matmul(zt_ps[:], RTr[:], hps[:], start=True,
                                 stop=True)
                zout = zpool.tile([16, B_LOC], F32, tag="z")
                nc.vector.tensor_copy(zout[:], zt_ps[:])
                nc.sync.dma_start(d_out.ap()[k], zout[:])

            HB = B_LOC // 2

            def substep(k, i, act_dt, w2a, w2b, w1sel, w1h):
                dX = dxs.pop((k, i))
                vfc_ps = ps1.tile([128, B_LOC], F32, tag="vfcp",
                                  name=f"vfcp_{k}_{i}")
                vfb_ps = ps1.tile([128, B_LOC], F32, tag="vfbp",
                                  name=f"vfbp_{k}_{i}")
                for h0, h1, s in ((0, HB, 0), (HB, B_LOC, 1)):
                    th = work.tile([128, HB], act_dt, tag=f"th{s}")
                    nc.scalar.activation(th[:], hpre[:, h0:h1], TANH,
                                         bias=b1c[:])
                    nc.tensor.matmul(vfc_ps[:, h0:h1], w2a[:], th[:],
                                     start=True, stop=True)
                    nc.tensor.matmul(vfb_ps[:, h0:h1], w2b[:], th[:],
                                     start=True, stop=True)
                    vfc = work.tile([128, HB], act_dt, tag=f"vfcs{s}")
                    nc.scalar.activation(vfc[:], vfc_ps[:, h0:h1], TANH,
                                         bias=b2c[:])
                    vfb = work.tile([128, HB], act_dt, tag=f"vfbs{s}")
                    nc.scalar.activation(vfb[:], vfb_ps[:, h0:h1], TANH,
                                         bias=b2t[:])
                    tmp = work.tile([128, HB], act_dt, tag=f"tmp{s}")
                    nc.vector.tensor_tensor(tmp[:], vfc[:], dX[:, h0:h1],
                                            MULT)
                    tp2 = work.tile([128, HB], act_dt, tag=f"tp2{s}")
                    nc.vector.tensor_tensor(tp2[:], tmp[:], vfb[:], ADD)
                    nc.tensor.matmul(hpre[:, h0:h1], w1sel[:], tp2[:],
                                     start=False, stop=False,
                                     skip_group_check=True)

            load_dX(0)
            load_dX(1)
            for k in range(n_intervals):
                load_dX(k + 2)
                if k < n32:
                    ws = (F32, W2aT, W2bT, W1SelT, W1hT)
                elif k < n32 + n16:
                    ws = (F16, W2aTh, W2bTh, W1SelTh, W1hTh)
                else:
                    ws = (BF16, W2aTw, W2bTw, W1SelTw, W1hTw)
                for i in range(4):
                    substep(k, i, *ws)
                out_interval(k)

    nc.compile()
    _BUILD_CACHE[key] = nc
    return nc


def _prep_core_inputs(us, ys, cst, core, n_intervals):
    import ml_dtypes
    n32, n16, nb8 = _band_sizes(n_intervals)
    b0 = core * B_LOC
    usc = np.ascontiguousarray(us[:, b0:b0 + B_LOC, :].transpose(0, 2, 1))  # (L,8,B)
    s = usc[1:] - usc[:-1]                                   # (L-1, 8, B)
    s_prev = np.concatenate([s[:1], s[:-1]], axis=0)         # backward diff
    # dX[k, i] = ALPHA[i]*s_prev[k] + BETA[i]*s[k]; rows r -> channel r//16+1
    al = np.array(ALPHA, dtype=np.float32)[None, :, None, None]
    be = np.array(BETA, dtype=np.float32)[None, :, None, None]
    dx = 8.0 * (al * s_prev[:n_intervals, None] + be * s[:n_intervals, None])
    dxb = np.repeat(dx, 16, axis=2).reshape(n_intervals * 4, 128, B_LOC)
    ys0T = np.ascontiguousarray(ys[0, b0:b0 + B_LOC, :].T).astype(np.float32)
    m = {"ys0T": ys0T}
    if n32:
        m["dXa"] = np.ascontiguousarray(dxb[:n32 * 4]).astype(np.float32)
    if n16:
        m["dXh"] = np.ascontiguousarray(
            dxb[n32 * 4:(n32 + n16) * 4]).astype(np.float16)
    if nb8:
        m["dXw"] = np.ascontiguousarray(
            dxb[(n32 + n16) * 4:]).astype(ml_dtypes.bfloat16)
    m.update({k: v for k, v in cst.items()
              if k not in ("W2bT", "b2t", "W2bRT", "b2tb")})
    m["W2bT"] = cst["W2bRT"]
    m["b2t"] = cst["b2tb"]
    for k in ("W2aT", "W1SelT", "W1hT"):
        m[k + "h"] = cst[k].astype(np.float16)
        m[k + "w"] = cst[k].astype(ml_dtypes.bfloat16)
    m["W2bTh"] = cst["W2bRT"].astype(np.float16)
    m["W2bTw"] = cst["W2bRT"].astype(ml_dtypes.bfloat16)
    return m


def kernel(ts, us, ys, W1, b1, W2, b2, batch_size=None, n_intervals=NI):
    from concourse.bass_utils import run_bass_kernel_spmd

    us = np.asarray(us, dtype=np.float32)
    ys = np.asarray(ys, dtype=np.float32)
    cst = _host_constants(np.asarray(W1, np.float32), np.asarray(b1, np.float32),
                          np.asarray(W2, np.float32), np.asarray(b2, np.float32))
    nc = _build(n_intervals)
    in_maps = [_prep_core_inputs(us, ys, cst, c, n_intervals) for c in range(N_CORES)]
    res = run_bass_kernel_spmd(nc, in_maps, core_ids=list(range(N_CORES)))
    out = np.empty((B_TOT, n_intervals + 1, Y), dtype=np.float32)
    out[:, 0, :] = ys[0]
    for c in range(N_CORES):
        b0 = c * B_LOC
        out[b0:b0 + B_LOC, 1:, :] = res.results[c]["out"].transpose(2, 0, 1)
    kernel._last_results = res
    return out


# revision 10
# speedup vs baseline: 1.4416x; 1.4416x over previous
"""Trainium2 Bass kernel for nn_GunnarODE: neural CDE with hermite spline control.

Contract: kernel(**inputs) takes FULL unsharded inputs (ts, us, ys, W1, b1,
W2, b2, batch_size) and returns the FULL (B, L, Y) output. Internally shards
the batch across 8 NeuronCores (pure data parallel), runs a Bass/Tile kernel
per core, and reassembles.

Algorithm notes (derived from the reference):
  - x = concat([t, us]) with unit-spaced knots (ts is arange) => dt == 1.
  - Hermite backward-difference spline derivative at substep s_i = i/4 of
    interval k reduces to dXdt_i = alpha_i * slope_{k-1} + beta_i * slope_k;
    the time channel has dXdt == 1. The 128-row broadcast of dXdt (row
    r=(c-1)*16+y -> channel c) is precomputed on the host and DMA'd, which
    removes one matmul + serial weight load per substep from the PE.
  - Per Euler substep: h = tanh(z@W1.T+b1); vf = tanh(h@W2.T+b2) viewed as
    (Y=16, C=9); z += 0.25 * einsum(vf, dXdt).
  - Feature-on-partition layout. The 144 vf rows split into 128 "ctrl" rows
    (r=(c-1)*16+y for channels c=1..8) and 16 "time" rows (y*9).
  - hpre = W1 @ z is THE state, held in a persistent fp32 PSUM accumulator;
    per substep it is incremented by two accumulating matmuls, and z is
    reconstructed per interval via R = pinv(W1) for output only.

Precision schedule (validated by a CPU error-amplification study + HW
microbenches):
  - The ODE is chaotic: a perturbation at interval k is amplified by G(k)
    (G(0)~2e4, G(128)~48, G(320)~4). Three bands: fp32 matmuls for k < 128,
    fp16 for 128 <= k < 320, bf16 for k >= 320 (fp32 PSUM accumulate
    everywhere). Simulated end-to-end rel-L2 vs the fp32 reference ~9.4e-3
    (budget 2e-2; the measured 2-band fp16 variant ran at 7.6e-3).
  - PE timing (HW-measured): fp32/fp16 matmuls are ~2 cyc/col; bf16 is
    ~1 cyc/col. Full-width (N=512) bodies beat half-split ones because the
    per-matmul serial LDWEIGHTS and fixed overheads amortize over 2x the
    columns.
"""
import sys
if '/opt/trn_rl_repo' not in sys.path:
    sys.path.insert(0, '/opt/trn_rl_repo')

import numpy as np

N_CORES = 8
L = 512
B_TOT = 4096
U = 8
Y = 16
H = 128
C = U + 1
NI = L - 1            # intervals
HSTEP = 0.25          # dt / SUBSTEPS with dt == 1
B_LOC = B_TOT // N_CORES  # 512
K0 = 128              # first fp16 interval
K1 = 320              # first bf16 interval

ALPHA = [1.0, 0.1875, -0.25, -0.3125]
BETA = [0.0, 0.8125, 1.25, 1.3125]

_BUILD_CACHE = {}


def _host_constants(W1, b1, W2, b2):
    """Precompute transposed/permuted constant matrices (host-side, free)."""
    rowmap = np.array([(r % 16) * 9 + (r // 16 + 1) for r in range(128)])
    cst = {}
    cst["W1T"] = np.ascontiguousarray(W1.T)                        # (16,128)
    cst["W2aT"] = np.ascontiguousarray(W2[rowmap, :].T)            # (128,128)
    cst["W2bT"] = np.ascontiguousarray(W2[np.arange(16) * 9, :].T)  # (128,16)
    cst["b1c"] = np.ascontiguousarray(b1[:, None])                 # (128,1)
    cst["b2c"] = np.ascontiguousarray(b2[rowmap][:, None])         # (128,1)
    cst["b2t"] = np.ascontiguousarray(b2[np.arange(16) * 9][:, None])  # (16,1)
    # replicated time-channel head: vfb[r] = vft[r % 16]
    rep = np.arange(128) % 16
    cst["W2bRT"] = np.ascontiguousarray(cst["W2bT"][:, rep])           # (128,128)
    cst["b2tb"] = np.ascontiguousarray(b2[np.arange(16) * 9][rep][:, None])
    # hpre-state update matrices: hpre += (h*W1*Sel^T) @ tmp + (h*W1) @ vft
    w1selt = np.zeros((128, 128), dtype=np.float32)  # [r, j] = h*W1[j, r%16]
    for r in range(128):
        w1selt[r, :] = HSTEP * W1[:, r % 16]
    cst["W1SelT"] = w1selt / 8.0  # dX is host-prescaled by 8; vfb unscaled
    cst["W1hT"] = (HSTEP * W1.T)                                   # (16,128)
    # output reconstruction: z = pinv(W1) @ hpre  (W1 is 128x16, cond ~2)
    R = np.linalg.pinv(W1.astype(np.float64)).astype(np.float32)   # (16,128)
    cst["RT"] = np.ascontiguousarray(R.T)                          # (128,16)
    return {k: v.astype(np.float32) for k, v in cst.items()}


def _band_sizes(n_intervals, k0=K0, k1=K1):
    n32 = min(k0, n_intervals)
    n16 = min(k1, n_intervals) - n32
    nb8 = n_intervals - n32 - n16
    return n32, n16, nb8


def _build(n_intervals=NI):
    """Build + compile the Bass module (cached per interval count)."""
    key = n_intervals
    if key in _BUILD_CACHE:
        return _BUILD_CACHE[key]

    import concourse.bass as bass
    import concourse.bacc as bacc
    import concourse.tile as tile
    from concourse import mybir

    F32 = mybir.dt.float32
    F32R = mybir.dt.float32r
    F16 = mybir.dt.float16
    BF16 = mybir.dt.bfloat16
    TANH = mybir.ActivationFunctionType.Tanh
    MULT = mybir.AluOpType.mult
    ADD = mybir.AluOpType.add

    n32, n16, nb8 = _band_sizes(n_intervals)

    nc = bacc.Bacc("TRN2", target_bir_lowering=False, debug=False,
                   num_devices=N_CORES)

    # host-precomputed 128-row dXdt broadcasts, one tensor per band
    if n32:
        d_dXa = nc.dram_tensor("dXa", (n32 * 4, 128, B_LOC), F32,
                               kind="ExternalInput")
    if n16:
        d_dXh = nc.dram_tensor("dXh", (n16 * 4, 128, B_LOC), F16,
                               kind="ExternalInput")
    if nb8:
        d_dXw = nc.dram_tensor("dXw", (nb8 * 4, 128, B_LOC), BF16,
                               kind="ExternalInput")
    d_ys0 = nc.dram_tensor("ys0T", (16, B_LOC), F32, kind="ExternalInput")
    d_W1T = nc.dram_tensor("W1T", (16, 128), F32, kind="ExternalInput")
    d_W2aT = nc.dram_tensor("W2aT", (128, 128), F32, kind="ExternalInput")
    d_W2bT = nc.dram_tensor("W2bT", (128, 128), F32, kind="ExternalInput")
    d_b1 = nc.dram_tensor("b1c", (128, 1), F32, kind="ExternalInput")
    d_b2c = nc.dram_tensor("b2c", (128, 1), F32, kind="ExternalInput")
    d_b2t = nc.dram_tensor("b2t", (128, 1), F32, kind="ExternalInput")
    d_W1SelT = nc.dram_tensor("W1SelT", (128, 128), F32, kind="ExternalInput")
    d_W1hT = nc.dram_tensor("W1hT", (16, 128), F32, kind="ExternalInput")
    d_RT = nc.dram_tensor("RT", (128, 16), F32, kind="ExternalInput")
    d_W2aTh = nc.dram_tensor("W2aTh", (128, 128), F16, kind="ExternalInput")
    d_W2bTh = nc.dram_tensor("W2bTh", (128, 128), F16, kind="ExternalInput")
    d_W1SelTh = nc.dram_tensor("W1SelTh", (128, 128), F16, kind="ExternalInput")
    d_W1hTh = nc.dram_tensor("W1hTh", (16, 128), F16, kind="ExternalInput")
    d_W2aTw = nc.dram_tensor("W2aTw", (128, 128), BF16, kind="ExternalInput")
    d_W2bTw = nc.dram_tensor("W2bTw", (128, 128), BF16, kind="ExternalInput")
    d_W1SelTw = nc.dram_tensor("W1SelTw", (128, 128), BF16, kind="ExternalInput")
    d_W1hTw = nc.dram_tensor("W1hTw", (16, 128), BF16, kind="ExternalInput")
    d_out = nc.dram_tensor("out", (n_intervals, 16, B_LOC), F32, kind="ExternalOutput")

    with tile.TileContext(nc) as tc:
        with (
            tc.tile_pool(name="consts", bufs=1) as consts,
            tc.tile_pool(name="zpool", bufs=3) as zpool,
            tc.tile_pool(name="work", bufs=2) as work,
            tc.tile_pool(name="dxp", bufs=3) as dxp,
            tc.tile_pool(name="ps1", bufs=1, space="PSUM") as ps1,
            tc.tile_pool(name="ps2", bufs=2, space="PSUM") as ps2,
        ):
            W1T = consts.tile([16, 128], F32)
            W2aT = consts.tile([128, 128], F32)
            W2bT = consts.tile([128, 128], F32)
            b1c = consts.tile([128, 1], F32)
            b2c = consts.tile([128, 1], F32)
            b2t = consts.tile([128, 1], F32)
            W1SelT = consts.tile([128, 128], F32)
            W1hT = consts.tile([16, 128], F32)
            RTr = consts.tile([128, 16], F32R)
            W2aTh = consts.tile([128, 128], F16)
            W2bTh = consts.tile([128, 128], F16)
            W1SelTh = consts.tile([128, 128], F16)
            W1hTh = consts.tile([16, 128], F16)
            W2aTw = consts.tile([128, 128], BF16)
            W2bTw = consts.tile([128, 128], BF16)
            W1SelTw = consts.tile([128, 128], BF16)
            W1hTw = consts.tile([16, 128], BF16)
            nc.sync.dma_start(W1T[:], d_W1T.ap())
            nc.sync.dma_start(W2aT[:], d_W2aT.ap())
            nc.sync.dma_start(W2bT[:], d_W2bT.ap())
            nc.sync.dma_start(b1c[:], d_b1.ap())
            nc.sync.dma_start(b2c[:], d_b2c.ap())
            nc.sync.dma_start(b2t[:], d_b2t.ap())
            nc.sync.dma_start(W1SelT[:], d_W1SelT.ap())
            nc.sync.dma_start(W1hT[:], d_W1hT.ap())
            nc.sync.dma_start(RTr[:], d_RT.ap().bitcast(F32R))
            nc.sync.dma_start(W2aTh[:], d_W2aTh.ap())
            nc.sync.dma_start(W2bTh[:], d_W2bTh.ap())
            nc.sync.dma_start(W1SelTh[:], d_W1SelTh.ap())
            nc.sync.dma_start(W1hTh[:], d_W1hTh.ap())
            nc.sync.dma_start(W2aTw[:], d_W2aTw.ap())
            nc.sync.dma_start(W2bTw[:], d_W2bTw.ap())
            nc.sync.dma_start(W1SelTw[:], d_W1SelTw.ap())
            nc.sync.dma_start(W1hTw[:], d_W1hTw.ap())

            z0 = zpool.tile([16, B_LOC], F32, tag="z")
            nc.sync.dma_start(z0[:], d_ys0.ap())

            # hpre is THE state: a persistent PSUM accumulator holding W1 @ z.
            hpre = ps1.tile([128, B_LOC], F32, tag="hpre")
            nc.tensor.matmul(hpre[:], W1T[:], z0[:], start=True, stop=False,
                             skip_group_check=True)

            dxs = {}

            def load_dX(k):
                """Prefetch the 4 substep dXdt tiles of interval k."""
                if k >= n_intervals:
                    return
                for i in range(4):
                    if k < n32:
                        t = dxp.tile([128, B_LOC], F32, tag=f"dxa{i}",
                                     name=f"dX_{k}_{i}")
                        nc.sync.dma_start(t[:], d_dXa.ap()[k * 4 + i])
                    elif k < n32 + n16:
                        t = dxp.tile([128, B_LOC], F16, tag=f"dxh{i}",
                                     name=f"dX_{k}_{i}")
                        nc.sync.dma_start(t[:], d_dXh.ap()[(k - n32) * 4 + i])
                    else:
                        t = dxp.tile([128, B_LOC], BF16, tag=f"dxw{i}",
                                     name=f"dX_{k}_{i}")
                        nc.sync.dma_start(t[:], d_dXw.ap()[(k - n32 - n16) * 4 + i])
                    dxs[(k, i)] = t

            def out_interval(k):
                # per-interval output: z_{k+1} = pinv(W1) @ hpre (fp32r is
                # fine: output tap only, no feedback into the state)
                hps = work.tile([128, B_LOC], F32R, tag="hps")
                nc.vector.tensor_copy(hps[:], hpre[:])
                zt_ps = ps2.tile([16, B_LOC], F32, tag="ztp")
                nc.tensor.matmul(zt_ps[:], RTr[:], hps[:], start=True,
                                 stop=True)
                zout = zpool.tile([16, B_LOC], F32, tag="z")
                nc.vector.tensor_copy(zout[:], zt_ps[:])
                nc.sync.dma_start(d_out.ap()[k], zout[:])

            def substep(k, i, act_dt, w2a, w2b, w1sel, w1h):
                dX = dxs.pop((k, i))
                th = work.tile([128, B_LOC], act_dt, tag="th0")
                nc.scalar.activation(th[:], hpre[:], TANH, bias=b1c[:])
                vfc_ps = ps1.tile([128, B_LOC], F32, tag="vfcp",
                                  name=f"vfcp_{k}_{i}")
                nc.tensor.matmul(vfc_ps[:], w2a[:], th[:], start=True,
                                 stop=True)
                vfb_ps = ps1.tile([128, B_LOC], F32, tag="vfbp",
                                  name=f"vfbp_{k}_{i}")
                nc.tensor.matmul(vfb_ps[:], w2b[:], th[:], start=True,
                                 stop=True)
                vfc = work.tile([128, B_LOC], act_dt, tag="vfcs0")
                nc.scalar.activation(vfc[:], vfc_ps[:], TANH, bias=b2c[:])
                vfb = work.tile([128, B_LOC], act_dt, tag="vfbs0")
                nc.scalar.activation(vfb[:], vfb_ps[:], TANH, bias=b2t[:])
                tmp = work.tile([128, B_LOC], act_dt, tag="tmp0")
                nc.vector.tensor_tensor(tmp[:], vfc[:], dX[:], MULT)
                tp2 = work.tile([128, B_LOC], act_dt, tag="tp20")
                nc.vector.tensor_tensor(tp2[:], tmp[:], vfb[:], ADD)
                nc.tensor.matmul(hpre[:], w1sel[:], tp2[:], start=False,
                                 stop=False, skip_group_check=True)

            load_dX(0)
            load_dX(1)
            for k in range(n_intervals):
                load_dX(k + 2)
                if k < n32:
                    ws = (F32, W2aT, W2bT, W1SelT, W1hT)
                elif k < n32 + n16:
                    ws = (F16, W2aTh, W2bTh, W1SelTh, W1hTh)
                else:
                    ws = (BF16, W2aTw, W2bTw, W1SelTw, W1hTw)
                for i in range(4):
                    substep(k, i, *ws)
                out_interval(k)

    nc.compile()
    _BUILD_CACHE[key] = nc
    return nc


def _prep_core_inputs(us, ys, cst, core, n_intervals):
    import ml_dtypes
    n32, n16, nb8 = _band_sizes(n_intervals)
    b0 = core * B_LOC
    usc = np.ascontiguousarray(us[:, b0:b0 + B_LOC, :].transpose(0, 2, 1))  # (L,8,B)
    s = usc[1:] - usc[:-1]                                   # (L-1, 8, B)
    s_prev = np.concatenate([s[:1], s[:-1]], axis=0)         # backward diff
    # dX[k, i] = ALPHA[i]*s_prev[k] + BETA[i]*s[k]; rows r -> channel r//16+1
    al = np.array(ALPHA, dtype=np.float32)[None, :, None, None]
    be = np.array(BETA, dtype=np.float32)[None, :, None, None]
    dx = 8.0 * (al * s_prev[:n_intervals, None] + be * s[:n_intervals, None])
    dxb = np.repeat(dx, 16, axis=2).reshape(n_intervals * 4, 128, B_LOC)
    ys0T = np.ascontiguousarray(ys[0, b0:b0 + B_LOC, :].T).astype(np.float32)
    m = {"ys0T": ys0T}
    if n32:
        m["dXa"] = np.ascontiguousarray(dxb[:n32 * 4]).astype(np.float32)
    if n16:
        m["dXh"] = np.ascontiguousarray(
            dxb[n32 * 4:(n32 + n16) * 4]).astype(np.float16)
    if nb8:
        m["dXw"] = np.ascontiguousarray(
            dxb[(n32 + n16) * 4:]).astype(ml_dtypes.bfloat16)
    m.update({k: v for k, v in cst.items()
              if k not in ("W2bT", "b2t", "W2bRT", "b2tb")})
    m["W2bT"] = cst["W2bRT"]
    m["b2t"] = cst["b2tb"]
    for k in ("W2aT", "W1SelT", "W1hT"):
        m[k + "h"] = cst[k].astype(np.float16)
        m[k + "w"] = cst[k].astype(ml_dtypes.bfloat16)
    m["W2bTh"] = cst["W2bRT"].astype(np.float16)
    m["W2bTw"] = cst["W2bRT"].astype(ml_dtypes.bfloat16)
    return m


def kernel(ts, us, ys, W1, b1, W2, b2, batch_size=None, n_intervals=NI):
    from concourse.bass_utils import run_bass_kernel_spmd

    us = np.asarray(us, dtype=np.float32)
    ys = np.asarray(ys, dtype=np.float32)
    cst = _host_constants(np.asarray(W1, np.float32), np.asarray(b1, np.float32),
                          np.asarray(W2, np.float32), np.asarray(b2, np.float32))
    nc = _build(n_intervals)
    in_maps = [_prep_core_inputs(us, ys, cst, c, n_intervals) for c in range(N_CORES)]
    res = run_bass_kernel_spmd(nc, in_maps, core_ids=list(range(N_CORES)))
    out = np.empty((B_TOT, n_intervals + 1, Y), dtype=np.float32)
    out[:, 0, :] = ys[0]
    for c in range(N_CORES):
        b0 = c * B_LOC
        out[b0:b0 + B_LOC, 1:, :] = res.results[c]["out"].transpose(2, 0, 1)
    kernel._last_results = res
    return out


# revision 14
# speedup vs baseline: 1.9732x; 1.3688x over previous
"""Trainium2 Bass kernel for nn_GunnarODE: neural CDE with hermite spline control.

Contract: kernel(**inputs) takes FULL unsharded inputs (ts, us, ys, W1, b1,
W2, b2, batch_size) and returns the FULL (B, L, Y) output. Internally shards
the batch across 8 NeuronCores (pure data parallel), runs a Bass/Tile kernel
per core, and reassembles.

Algorithm notes (derived from the reference):
  - x = concat([t, us]) with unit-spaced knots (ts is arange) => dt == 1.
  - Hermite backward-difference spline derivative at substep s_i = i/4 of
    interval k reduces to dXdt_i = alpha_i * slope_{k-1} + beta_i * slope_k;
    the time channel has dXdt == 1. The 128-row broadcast of dXdt (row
    r=(c-1)*16+y -> channel c) is precomputed on the host and DMA'd, which
    removes one matmul + serial weight load per substep from the PE.
  - Per Euler substep: h = tanh(z@W1.T+b1); vf = tanh(h@W2.T+b2) viewed as
    (Y=16, C=9); z += 0.25 * einsum(vf, dXdt).
  - Feature-on-partition layout. The 144 vf rows split into 128 "ctrl" rows
    (r=(c-1)*16+y for channels c=1..8) and 16 "time" rows (y*9).
  - hpre = W1 @ z is THE state, held in a persistent fp32 PSUM accumulator;
    per substep it is incremented by two accumulating matmuls, and z is
    reconstructed per interval via R = pinv(W1) for output only.

Precision schedule (validated by a CPU error-amplification study + HW
microbenches):
  - The ODE is chaotic: a perturbation at interval k is amplified by G(k)
    (G(0)~2e4, G(128)~48, G(320)~4). Three bands: fp32 matmuls for k < 128,
    fp16 for 128 <= k < 320, bf16 for k >= 320 (fp32 PSUM accumulate
    everywhere). Simulated end-to-end rel-L2 vs the fp32 reference ~9.4e-3
    (budget 2e-2; the measured 2-band fp16 variant ran at 7.6e-3).
  - PE timing (HW-measured): fp32/fp16 matmuls are ~2 cyc/col; bf16 is
    ~1 cyc/col. Full-width (N=512) bodies beat half-split ones because the
    per-matmul serial LDWEIGHTS and fixed overheads amortize over 2x the
    columns.
"""
import sys
if '/opt/trn_rl_repo' not in sys.path:
    sys.path.insert(0, '/opt/trn_rl_repo')

import numpy as np

N_CORES = 8
L = 512
B_TOT = 4096
U = 8
Y = 16
H = 128
C = U + 1
NI = L - 1            # intervals
HSTEP = 0.25          # dt / SUBSTEPS with dt == 1
B_LOC = B_TOT // N_CORES  # 512
K0 = 128              # first fp16 interval
K1 = 320              # first bf16 interval

ALPHA = [1.0, 0.1875, -0.25, -0.3125]
BETA = [0.0, 0.8125, 1.25, 1.3125]

_BUILD_CACHE = {}


def _host_constants(W1, b1, W2, b2):
    """Precompute transposed/permuted constant matrices (host-side, free)."""
    rowmap = np.array([(r % 16) * 9 + (r // 16 + 1) for r in range(128)])
    cst = {}
    cst["W1T"] = np.ascontiguousarray(W1.T)                        # (16,128)
    cst["W2aT"] = np.ascontiguousarray(W2[rowmap, :].T)            # (128,128)
    cst["W2bT"] = np.ascontiguousarray(W2[np.arange(16) * 9, :].T)  # (128,16)
    cst["b1c"] = np.ascontiguousarray(b1[:, None])                 # (128,1)
    cst["b2c"] = np.ascontiguousarray(b2[rowmap][:, None])         # (128,1)
    cst["b2t"] = np.ascontiguousarray(b2[np.arange(16) * 9][:, None])  # (16,1)
    # replicated time-channel head: vfb[r] = vft[r % 16]
    rep = np.arange(128) % 16
    cst["W2bRT"] = np.ascontiguousarray(cst["W2bT"][:, rep])           # (128,128)
    cst["b2tb"] = np.ascontiguousarray(b2[np.arange(16) * 9][rep][:, None])
    # hpre-state update matrices: hpre += (h*W1*Sel^T) @ tmp + (h*W1) @ vft
    w1selt = np.zeros((128, 128), dtype=np.float32)  # [r, j] = h*W1[j, r%16]
    for r in range(128):
        w1selt[r, :] = HSTEP * W1[:, r % 16]
    cst["W1SelT"] = w1selt / 8.0  # dX is host-prescaled by 8; vfb unscaled
    cst["W1hT"] = (HSTEP * W1.T)                                   # (16,128)
    # output reconstruction: z = pinv(W1) @ hpre  (W1 is 128x16, cond ~2)
    R = np.linalg.pinv(W1.astype(np.float64)).astype(np.float32)   # (16,128)
    cst["RT"] = np.ascontiguousarray(R.T)                          # (128,16)
    return {k: v.astype(np.float32) for k, v in cst.items()}


def _band_sizes(n_intervals, k0=K0, k1=K1):
    n32 = min(k0, n_intervals)
    n16 = min(k1, n_intervals) - n32
    nb8 = n_intervals - n32 - n16
    return n32, n16, nb8


def _build(n_intervals=NI):
    """Build + compile the Bass module (cached per interval count)."""
    key = n_intervals
    if key in _BUILD_CACHE:
        return _BUILD_CACHE[key]

    import concourse.bass as bass
    import concourse.bacc as bacc
    import concourse.tile as tile
    from concourse import mybir

    F32 = mybir.dt.float32
    F32R = mybir.dt.float32r
    F16 = mybir.dt.float16
    BF16 = mybir.dt.bfloat16
    TANH = mybir.ActivationFunctionType.Tanh
    MULT = mybir.AluOpType.mult
    ADD = mybir.AluOpType.add

    n32, n16, nb8 = _band_sizes(n_intervals)

    nc = bacc.Bacc("TRN2", target_bir_lowering=False, debug=False,
                   num_devices=N_CORES)

    # host-precomputed 128-row dXdt broadcasts, one tensor per band
    if n32:
        d_dXa = nc.dram_tensor("dXa", (n32 * 4, 128, B_LOC), F32,
                               kind="ExternalInput")
    if n16:
        d_dXh = nc.dram_tensor("dXh", (n16 * 4, 128, B_LOC), F16,
                               kind="ExternalInput")
    if nb8:
        d_dXw = nc.dram_tensor("dXw", (nb8 * 4, 128, B_LOC), BF16,
                               kind="ExternalInput")
    d_ys0 = nc.dram_tensor("ys0T", (16, B_LOC), F32, kind="ExternalInput")
    d_W1T = nc.dram_tensor("W1T", (16, 128), F32, kind="ExternalInput")
    d_W2aT = nc.dram_tensor("W2aT", (128, 128), F32, kind="ExternalInput")
    d_W2bT = nc.dram_tensor("W2bT", (128, 128), F32, kind="ExternalInput")
    d_b1 = nc.dram_tensor("b1c", (128, 1), F32, kind="ExternalInput")
    d_b2c = nc.dram_tensor("b2c", (128, 1), F32, kind="ExternalInput")
    d_b2t = nc.dram_tensor("b2t", (128, 1), F32, kind="ExternalInput")
    d_W1SelT = nc.dram_tensor("W1SelT", (128, 128), F32, kind="ExternalInput")
    d_W1hT = nc.dram_tensor("W1hT", (16, 128), F32, kind="ExternalInput")
    d_RT = nc.dram_tensor("RT", (128, 16), F32, kind="ExternalInput")
    d_W2aTh = nc.dram_tensor("W2aTh", (128, 128), F16, kind="ExternalInput")
    d_W2bTh = nc.dram_tensor("W2bTh", (128, 128), F16, kind="ExternalInput")
    d_W1SelTh = nc.dram_tensor("W1SelTh", (128, 128), F16, kind="ExternalInput")
    d_W1hTh = nc.dram_tensor("W1hTh", (16, 128), F16, kind="ExternalInput")
    d_W2aTw = nc.dram_tensor("W2aTw", (128, 128), BF16, kind="ExternalInput")
    d_W2bTw = nc.dram_tensor("W2bTw", (128, 128), BF16, kind="ExternalInput")
    d_W1SelTw = nc.dram_tensor("W1SelTw", (128, 128), BF16, kind="ExternalInput")
    d_W1hTw = nc.dram_tensor("W1hTw", (16, 128), BF16, kind="ExternalInput")
    d_out = nc.dram_tensor("out", (n_intervals, 16, B_LOC), F32, kind="ExternalOutput")

    with tile.TileContext(nc) as tc:
        with (
            tc.tile_pool(name="consts", bufs=1) as consts,
            tc.tile_pool(name="zpool", bufs=3) as zpool,
            tc.tile_pool(name="work", bufs=2) as work,
            tc.tile_pool(name="dxp", bufs=3) as dxp,
            tc.tile_pool(name="ps1", bufs=1, space="PSUM") as ps1,
            tc.tile_pool(name="ps2", bufs=2, space="PSUM") as ps2,
        ):
            W1T = consts.tile([16, 128], F32)
            W2aT = consts.tile([128, 128], F32)
            W2bT = consts.tile([128, 128], F32)
            b1c = consts.tile([128, 1], F32)
            b2c = consts.tile([128, 1], F32)
            b2t = consts.tile([128, 1], F32)
            W1SelT = consts.tile([128, 128], F32)
            W1hT = consts.tile([16, 128], F32)
            RTr = consts.tile([128, 16], F32R)
            W2aTh = consts.tile([128, 128], F16)
            W2bTh = consts.tile([128, 128], F16)
            W1SelTh = consts.tile([128, 128], F16)
            W1hTh = consts.tile([16, 128], F16)
            W2aTw = consts.tile([128, 128], BF16)
            W2bTw = consts.tile([128, 128], BF16)
            W1SelTw = consts.tile([128, 128], BF16)
            W1hTw = consts.tile([16, 128], BF16)
            nc.sync.dma_start(W1T[:], d_W1T.ap())
            nc.sync.dma_start(W2aT[:], d_W2aT.ap())
            nc.sync.dma_start(W2bT[:], d_W2bT.ap())
            nc.sync.dma_start(b1c[:], d_b1.ap())
            nc.sync.dma_start(b2c[:], d_b2c.ap())
            nc.sync.dma_start(b2t[:], d_b2t.ap())
            nc.sync.dma_start(W1SelT[:], d_W1SelT.ap())
            nc.sync.dma_start(W1hT[:], d_W1hT.ap())
            nc.sync.dma_start(RTr[:], d_RT.ap().bitcast(F32R))
            nc.sync.dma_start(W2aTh[:], d_W2aTh.ap())
            nc.sync.dma_start(W2bTh[:], d_W2bTh.ap())
            nc.sync.dma_start(W1SelTh[:], d_W1SelTh.ap())
            nc.sync.dma_start(W1hTh[:], d_W1hTh.ap())
            nc.sync.dma_start(W2aTw[:], d_W2aTw.ap())
            nc.sync.dma_start(W2bTw[:], d_W2bTw.ap())
            nc.sync.dma_start(W1SelTw[:], d_W1SelTw.ap())
            nc.sync.dma_start(W1hTw[:], d_W1hTw.ap())

            z0 = zpool.tile([16, B_LOC], F32, tag="z")
            nc.sync.dma_start(z0[:], d_ys0.ap())

            # hpre is THE state: W1 @ z in persistent PSUM accumulators.
            # One tile PER half-batch stream so the two streams have fully
            # independent dependency chains.
            HB = B_LOC // 2
            hpreS = [ps1.tile([128, HB], F32, tag=f"hpre{s}", name=f"hpre{s}")
                     for s in (0, 1)]
            for s in (0, 1):
                nc.tensor.matmul(hpreS[s][:], W1T[:], z0[:, s*HB:(s+1)*HB],
                                 start=True, stop=False, skip_group_check=True)

            dxs = {}

            def load_dX(k):
                """Prefetch the 4 substep dXdt tiles of interval k."""
                if k >= n_intervals:
                    return
                for i in range(4):
                    if k < n32:
                        t = dxp.tile([128, B_LOC], F32, tag=f"dxa{i}",
                                     name=f"dX_{k}_{i}")
                        nc.sync.dma_start(t[:], d_dXa.ap()[k * 4 + i])
                    elif k < n32 + n16:
                        t = dxp.tile([128, B_LOC], F16, tag=f"dxh{i}",
                                     name=f"dX_{k}_{i}")
                        nc.sync.dma_start(t[:], d_dXh.ap()[(k - n32) * 4 + i])
                    else:
                        t = dxp.tile([128, B_LOC], BF16, tag=f"dxw{i}",
                                     name=f"dX_{k}_{i}")
                        nc.sync.dma_start(t[:], d_dXw.ap()[(k - n32 - n16) * 4 + i])
                    dxs[(k, i)] = t

            def out_interval(k):
                # per-interval output: z_{k+1} = pinv(W1) @ hpre (fp32r is
                # fine: output tap only, no feedback into the state)
                hps = work.tile([128, B_LOC], F32R, tag="hps")
                nc.vector.tensor_copy(hps[:, :HB], hpreS[0][:])
                nc.vector.tensor_copy(hps[:, HB:], hpreS[1][:])
                zt_ps = ps2.tile([16, B_LOC], F32, tag="ztp")
                nc.tensor.matmul(zt_ps[:], RTr[:], hps[:], start=True,
                                 stop=True)
                zout = zpool.tile([16, B_LOC], F32, tag="z")
                nc.vector.tensor_copy(zout[:], zt_ps[:])
                nc.sync.dma_start(d_out.ap()[k], zout[:])

            def substep(k, i, act_dt, w2a, w2b, w1sel, w1h):
                dX = dxs.pop((k, i))
                for s in (0, 1):
                    h0 = s * HB
                    hp = hpreS[s]
                    th = work.tile([128, HB], act_dt, tag=f"th{s}")
                    nc.scalar.activation(th[:], hp[:], TANH, bias=b1c[:])
                    vfc_ps = ps1.tile([128, HB], F32, tag=f"vfcp{s}",
                                      name=f"vfcp{s}_{k}_{i}")
                    nc.tensor.matmul(vfc_ps[:], w2a[:], th[:], start=True,
                                     stop=True)
                    vfb_ps = ps1.tile([128, HB], F32, tag=f"vfbp{s}",
                                      name=f"vfbp{s}_{k}_{i}")
                    nc.tensor.matmul(vfb_ps[:], w2b[:], th[:], start=True,
                                     stop=True)
                    vfc = work.tile([128, HB], act_dt, tag=f"vfcs{s}")
                    nc.scalar.activation(vfc[:], vfc_ps[:], TANH, bias=b2c[:])
                    vfb = work.tile([128, HB], act_dt, tag=f"vfbs{s}")
                    nc.scalar.activation(vfb[:], vfb_ps[:], TANH, bias=b2t[:])
                    tmp = work.tile([128, HB], act_dt, tag=f"tmp{s}")
                    nc.vector.tensor_tensor(tmp[:], vfc[:], dX[:, h0:h0 + HB],
                                            MULT)
                    tp2 = work.tile([128, HB], act_dt, tag=f"tp2{s}")
                    nc.vector.tensor_tensor(tp2[:], tmp[:], vfb[:], ADD)
                    nc.tensor.matmul(hp[:], w1sel[:], tp2[:], start=False,
                                     stop=False, skip_group_check=True)

            load_dX(0)
            load_dX(1)
            for k in range(n_intervals):
                load_dX(k + 2)
                if k < n32:
                    ws = (F32, W2aT, W2bT, W1SelT, W1hT)
                elif k < n32 + n16:
                    ws = (F16, W2aTh, W2bTh, W1SelTh, W1hTh)
                else:
                    ws = (BF16, W2aTw, W2bTw, W1SelTw, W1hTw)
                for i in range(4):
                    substep(k, i, *ws)
                out_interval(k)

    nc.compile()
    _BUILD_CACHE[key] = nc
    return nc


def _prep_core_inputs(us, ys, cst, core, n_intervals):
    import ml_dtypes
    n32, n16, nb8 = _band_sizes(n_intervals)
    b0 = core * B_LOC
    usc = np.ascontiguousarray(us[:, b0:b0 + B_LOC, :].transpose(0, 2, 1))  # (L,8,B)
    s = usc[1:] - usc[:-1]                                   # (L-1, 8, B)
    s_prev = np.concatenate([s[:1], s[:-1]], axis=0)         # backward diff
    # dX[k, i] = ALPHA[i]*s_prev[k] + BETA[i]*s[k]; rows r -> channel r//16+1
    al = np.array(ALPHA, dtype=np.float32)[None, :, None, None]
    be = np.array(BETA, dtype=np.float32)[None, :, None, None]
    dx = 8.0 * (al * s_prev[:n_intervals, None] + be * s[:n_intervals, None])
    dxb = np.repeat(dx, 16, axis=2).reshape(n_intervals * 4, 128, B_LOC)
    ys0T = np.ascontiguousarray(ys[0, b0:b0 + B_LOC, :].T).astype(np.float32)
    m = {"ys0T": ys0T}
    if n32:
        m["dXa"] = np.ascontiguousarray(dxb[:n32 * 4]).astype(np.float32)
    if n16:
        m["dXh"] = np.ascontiguousarray(
            dxb[n32 * 4:(n32 + n16) * 4]).astype(np.float16)
    if nb8:
        m["dXw"] = np.ascontiguousarray(
            dxb[(n32 + n16) * 4:]).astype(ml_dtypes.bfloat16)
    m.update({k: v for k, v in cst.items()
              if k not in ("W2bT", "b2t", "W2bRT", "b2tb")})
    m["W2bT"] = cst["W2bRT"]
    m["b2t"] = cst["b2tb"]
    for k in ("W2aT", "W1SelT", "W1hT"):
        m[k + "h"] = cst[k].astype(np.float16)
        m[k + "w"] = cst[k].astype(ml_dtypes.bfloat16)
    m["W2bTh"] = cst["W2bRT"].astype(np.float16)
    m["W2bTw"] = cst["W2bRT"].astype(ml_dtypes.bfloat16)
    return m


def kernel(ts, us, ys, W1, b1, W2, b2, batch_size=None, n_intervals=NI):
    from concourse.bass_utils import run_bass_kernel_spmd

    us = np.asarray(us, dtype=np.float32)
    ys = np.asarray(ys, dtype=np.float32)
    cst = _host_constants(np.asarray(W1, np.float32), np.asarray(b1, np.float32),
                          np.asarray(W2, np.float32), np.asarray(b2, np.float32))
    nc = _build(n_intervals)
    in_maps = [_prep_core_inputs(us, ys, cst, c, n_intervals) for c in range(N_CORES)]
    res = run_bass_kernel_spmd(nc, in_maps, core_ids=list(range(N_CORES)))
    out = np.empty((B_TOT, n_intervals + 1, Y), dtype=np.float32)
    out[:, 0, :] = ys[0]
    for c in range(N_CORES):
        b0 = c * B_LOC
        out[b0:b0 + B_LOC, 1:, :] = res.results[c]["out"].transpose(2, 0, 1)
    kernel._last_results = res
    return out
